# revision 1
# baseline (speedup 1.0000x reference)
"""Trainium2 Bass kernel for CustomGRU (B=64, T=512, D=512, U=1024).

Sharding: data-parallel over batch across 8 NeuronCores (8 rows each),
weights replicated (a per-step inter-core exchange is ruled out by the
~5-12us collective latency floor x 512 steps). Per core:

  Phase 1 (projections): xzr[t,b,:] = X[b,t,:] @ [Wz|Wr|Wh] + b  -> DRAM
    - stationary = X^T tiles (host-pre-transposed), moving = Wcat (f32r,
      1 cyc/row at N=512).
  Phase 2 (recurrence), per step t, all in B-major [8, u] except the
  matmul stationaries:
    - gate pre-activations h_{t-1} @ U via 4-way column-tiled PE
      streaming: h^T chunks [128,8] (zero-padded to M=32 slots) are
      stationary in four 32-column groups of the PE array
      (tile_position=(0,32g)); the fp16 U-weight slices [128,512] stream
      through 4 XBUSes concurrently, 2 rounds of 4 chunks accumulating
      into partition blocks 32g..32g+8 of one PSUM tile. Round-1 matmuls
      use start=True (the has_written clear is region-scoped). An
      "eye-matmul" accumulates xzr_t (kept f32r for precision) onto
      group 0. A copy + ones-pattern matmul reduces the 4 partition
      blocks to the [8,512] gate pre-activation.
    - sigmoid/tanh on ScalarE evict the reduced PSUM -> SBUF.
    - r is PE-transposed to U-major to form (r*h)^T, the stationary of
      the candidate matmul; h_new is PE-transposed back to h^T.
    - combine h = hh + z*(h_prev - hh) on VectorE; the tanh/combine/
      transpose/copy tail is split into 512-halves so the next step's
      round-1 matmuls (needing only h chunks 0-3) start early.

Weight matmuls run in fp16 (1 cyc/row, col-tiling compatible; ~2e-4
end-to-end rel err, same order as float32r); reductions, xz preloads
and projections in f32r; everything else fp32.
"""
import sys

if "/opt/trn_rl_repo" not in sys.path:
    sys.path.insert(0, "/opt/trn_rl_repo")

import numpy as np
from contextlib import ExitStack

import concourse.bass as bass
import concourse.bacc as bacc
import concourse.tile as tile
from concourse import mybir
from concourse.bass_utils import run_bass_kernel_spmd

F32 = mybir.dt.float32
F32R = mybir.dt.float32r
F16 = mybir.dt.float16

N_CORES = 8
B = 64
BS = B // N_CORES  # 8 batch rows per core
D = 512
U = 1024
U3 = 3 * U        # 3072 (z|r|h)
KC = U // 128     # 8 contraction chunks of 128
DC = D // 128     # 4 input-dim chunks


def build(nc, T, reps=1):
    BT = BS * T

    # ---- DRAM I/O (per-core) ----
    xT_d = nc.dram_tensor("xT", [D, BT], F32R, kind="ExternalInput")
    wcat_d = nc.dram_tensor("wcat", [D, U3], F32R, kind="ExternalInput")
    bb_d = nc.dram_tensor("bb", [128, U3], F32, kind="ExternalInput")
    uzr_d = nc.dram_tensor("uzr", [U, 2 * U], F16, kind="ExternalInput")
    uh_d = nc.dram_tensor("uh", [U, U], F16, kind="ExternalInput")
    eye8r_d = nc.dram_tensor("eye8r", [BS, BS], F32R, kind="ExternalInput")
    ones4_d = nc.dram_tensor("ones4", [128, BS], F32R, kind="ExternalInput")
    eye8f_d = nc.dram_tensor("eye8f", [BS, BS], F32, kind="ExternalInput")
    out_d = nc.dram_tensor("out", [T, BS, U], F32, kind="ExternalOutput")

    with tile.TileContext(nc) as tc, ExitStack() as ctx:
        dram = ctx.enter_context(tc.tile_pool(name="dram", bufs=1, space="DRAM"))
        xzr_d = dram.tile([T, BS, U3], F32R)

        const = ctx.enter_context(tc.tile_pool(name="const", bufs=1))
        eye8r = const.tile([BS, BS], F32R)
        nc.sync.dma_start(eye8r[:], eye8r_d[:])
        ones4 = const.tile([128, BS], F32R)
        nc.sync.dma_start(ones4[:], ones4_d[:])
        eye8f = const.tile([BS, BS], F32)
        nc.sync.dma_start(eye8f[:], eye8f_d[:])

        # ---------------- Phase 1: input projections ----------------
        with ExitStack() as p1:
            wpool = p1.enter_context(tc.tile_pool(name="wcat", bufs=1))
            wcat = wpool.tile([128, DC * U3], F32R)  # [p, dc, u]
            nc.sync.dma_start(
                wcat[:].rearrange("p (dc u) -> p dc u", dc=DC),
                wcat_d.rearrange("(dc p) u -> p dc u", p=128),
            )
            bb = wpool.tile([128, U3], F32)
            nc.sync.dma_start(bb[:], bb_d[:])

            xp = p1.enter_context(tc.tile_pool(name="xT", bufs=3))
            op = p1.enter_context(tc.tile_pool(name="p1out", bufs=3))
            pp = p1.enter_context(tc.tile_pool(name="p1ps", bufs=4, space="PSUM"))

            n_bt = BT // 128          # bt-chunks of 128 (4 per batch row)
            tpb = T // 128            # t-chunks per batch row
            for tb in range(n_bt):
                b_idx, t_blk = tb // tpb, tb % tpb
                xt = xp.tile([128, DC * 128], F32R, tag="xt")  # [p=d, dc, bt]
                nc.sync.dma_start(
                    xt[:].rearrange("p (dc n) -> p dc n", dc=DC),
                    xT_d[:, tb * 128:(tb + 1) * 128].rearrange(
                        "(dc p) n -> p dc n", p=128
                    ),
                )
                for ut in range(U3 // 512):
                    ps = pp.tile([128, 512], F32, tag="ps")
                    for dc in range(DC):
                        nc.tensor.matmul(
                            ps[:],
                            xt[:, dc * 128:(dc + 1) * 128],
                            wcat[:, dc * U3 + ut * 512: dc * U3 + ut * 512 + 512],
                            start=(dc == 0),
                            stop=(dc == DC - 1),
                        )
                    ob = op.tile([128, 512], F32R, tag="ob")
                    nc.vector.tensor_add(
                        ob[:], ps[:], bb[:, ut * 512:(ut + 1) * 512]
                    )
                    nc.sync.dma_start(
                        xzr_d[
                            t_blk * 128:(t_blk + 1) * 128,
                            b_idx,
                            ut * 512:(ut + 1) * 512,
                        ].squeeze(),
                        ob[:],
                    )

        # ---------------- Phase 2: recurrence ----------------
        upool = ctx.enter_context(tc.tile_pool(name="u", bufs=1))
        uzr = upool.tile([128, KC * 2 * U], F16)  # [p, k, 2U]
        nc.sync.dma_start(
            uzr[:].rearrange("p (k u) -> p k u", k=KC),
            uzr_d.rearrange("(k p) u -> p k u", p=128),
        )
        uh = upool.tile([128, KC * U], F16)
        nc.sync.dma_start(
            uh[:].rearrange("p (k u) -> p k u", k=KC),
            uh_d.rearrange("(k p) u -> p k u", p=128),
        )

        hpool = ctx.enter_context(tc.tile_pool(name="h", bufs=2))
        stage = ctx.enter_context(tc.tile_pool(name="stage", bufs=4))
        gates = ctx.enter_context(tc.tile_pool(name="gates", bufs=2))
        psg = ctx.enter_context(tc.tile_pool(name="psg", bufs=4, space="PSUM"))
        ps2 = ctx.enter_context(tc.tile_pool(name="ps2", bufs=2, space="PSUM"))
        pst = ctx.enter_context(tc.tile_pool(name="pst", bufs=1, space="PSUM"))
        red = ctx.enter_context(tc.tile_pool(name="red", bufs=3))
        # zero the col-tiled psum slots once so untouched partition rows
        # (multiplied by 0 in the ones-reduction) are never uninitialized
        for _i in range(4):
            _d = psg.tile([128, 512], F32, tag="psg")
            nc.vector.memset(_d[:], 0.0)

        # h^T chunks in 32-col padded slots (cols 32k..32k+8 hold chunk k,
        # rest zero) so col-tiled matmuls use M=32 stationaries.
        hT0 = const.tile([128, KC * 32], F16)
        nc.any.memzero(hT0[:])
        hT_prev = hT0
        # pre-zero the rT/hTps psum slots once: per-step transposes write
        # only the 8 valid cols of each 32-col slot; the full-width mul/copy
        # reads the (zero) pads
        _c = pst.tile([128, KC * 32], F32, tag="rT")
        nc.vector.memset(_c[:], 0.0)
        _e = pst.tile([128, KC * 32], F32, tag="hTps")
        nc.vector.memset(_e[:], 0.0)

        def gate_mms(xoff, uoff, umat, hT, tag, copy_eng):
            """Two [8,512] reduced psum tiles via 4-way col-tiled streaming.

            Each [8,512] gate tile: eye-MM preloads xz into partitions 0-8
            (start=True clears the bank), then 8 K-chunk matmuls run on 4
            col-groups (tile_position=(0,32g), 2 rounds) writing partials to
            partition blocks 32g..32g+8. A copy + ones-pattern matmul sums
            the 4 blocks (+xz) back to [8,512]."""
            tiles = []
            for j in range(2):
                ps = psg.tile([128, 512], F32, tag="psg")
                xz = xz_t[:, xoff + 512 * j: xoff + 512 * j + 512]
                # round 1 (chunks 0-3): start=True so each group clears its
                # own 32-row psum region (has_written clear is region-scoped)
                for k in range(KC):
                    g = k % 4
                    nc.tensor.matmul(
                        ps[32 * g:32 * g + 32, :],
                        hT[:, k * 32:(k + 1) * 32],
                        umat[:, k * WSTRIDE + uoff + 512 * j:
                             k * WSTRIDE + uoff + 512 * j + 512],
                        start=(k < 4),
                        stop=(k == KC - 1),
                        tile_position=(0, 32 * g),
                        skip_group_check=True,
                    )
                    if k == 3:
                        # xz preload accumulates onto group 0's rows 0-8
                        nc.tensor.matmul(ps[0:BS, :], eye8r[:], xz,
                                         start=False, stop=False,
                                         tile_position=(0, 0),
                                         skip_group_check=True)
                sb = red.tile([128, 512], F32R, tag="red")
                if copy_eng == "act":
                    nc.scalar.copy(sb[:], ps[:])
                else:
                    nc.vector.tensor_copy(sb[:], ps[:])
                pr = ps2.tile([BS, 512], F32, tag="ps2")
                nc.tensor.matmul(pr[:], ones4[:], sb[:], start=True, stop=True)
                tiles.append(pr)
            return tiles

        for rep in range(reps):
          for t in range(T):
            xz_t = stage.tile([BS, U3], F32R, tag="xz")
            nc.sync.dma_start(xz_t[:], xzr_d[t].squeeze())

            # r gate, then transpose to U-major and form (r*h)^T
            WSTRIDE = 2 * U
            ps_r = gate_mms(U, U, uzr, hT_prev, "r", "dve")
            # z gate (keeps PE busy while sigmoid(r) runs)
            ps_z = gate_mms(0, 0, uzr, hT_prev, "z", "act")
            r_B = gates.tile([BS, U], F32, tag="r")
            rT = pst.tile([128, KC * 32], F32, tag="rT")
            rhT = hpool.tile([128, KC * 32], F16, tag="rhT")
            for j in range(2):
                sl = slice(512 * j, 512 * j + 512)
                nc.scalar.activation(
                    r_B[:, sl], ps_r[j][:],
                    mybir.ActivationFunctionType.Sigmoid,
                )
                for c in range(4 * j, 4 * j + 4):
                    nc.tensor.transpose(
                        rT[:, c * 32:c * 32 + BS],
                        r_B[:, c * 128:(c + 1) * 128],
                        eye8f[:],
                    )
                nc.vector.tensor_mul(
                    rhT[:, 128 * j:128 * (j + 1)],
                    rT[:, 128 * j:128 * (j + 1)],
                    hT_prev[:, 128 * j:128 * (j + 1)])
            z_B = gates.tile([BS, U], F32, tag="z")
            for j in range(2):
                nc.scalar.activation(
                    z_B[:, 512 * j:512 * j + 512], ps_z[j][:],
                    mybir.ActivationFunctionType.Sigmoid,
                )

            # candidate
            WSTRIDE = U
            ps_h = gate_mms(2 * U, 0, uh, rhT, "hh", "act")
            if t == 0:
                h_B_prev = gates.tile([BS, U], F32, tag="hB")
                nc.any.memzero(h_B_prev[:])
            hh_B = gates.tile([BS, U], F32, tag="hh")
            h_B = gates.tile([BS, U], F32, tag="hB")
            hT_ps = pst.tile([128, KC * 32], F32, tag="hTps")
            hT_new = hpool.tile([128, KC * 32], F16, tag="hT")
            # per 512-half: tanh -> combine -> transpose -> h^T copy, so the
            # next step's round-1 matmuls (which read only h chunks 0-3)
            # start while this half-1 is still combining
            for j in range(2):
                sl = slice(512 * j, 512 * j + 512)
                nc.scalar.activation(
                    hh_B[:, sl], ps_h[j][:],
                    mybir.ActivationFunctionType.Tanh,
                )
                # combine: h = hh + z * (h_prev - hh)   (B-major, VectorE)
                tmp = gates.tile([BS, 512], F32, tag="tmp")
                nc.vector.tensor_sub(tmp[:], h_B_prev[:, sl], hh_B[:, sl])
                nc.vector.tensor_mul(tmp[:], z_B[:, sl], tmp[:])
                nc.vector.tensor_add(h_B[:, sl], hh_B[:, sl], tmp[:])
                for c in range(4 * j, 4 * j + 4):
                    nc.tensor.transpose(
                        hT_ps[:, c * 32:c * 32 + BS],
                        h_B[:, c * 128:(c + 1) * 128],
                        eye8f[:],
                    )
                nc.vector.tensor_copy(
                    hT_new[:, 128 * j:128 * (j + 1)],
                    hT_ps[:, 128 * j:128 * (j + 1)])

            nc.sync.dma_start(out_d[t].squeeze(), h_B[:])
            hT_prev = hT_new
            h_B_prev = h_B

    nc.compile()
    return nc


def prepare(inputs, Wz, Uz, bz, Wr, Ur, br, Wh, Uh, bh, T):
    """Build the Bass program and the per-core input maps."""
    x = np.asarray(inputs, dtype=np.float32)[:, :T, :]

    wcat = np.concatenate([Wz, Wr, Wh], axis=1).astype(np.float32)
    bcat = np.concatenate([bz, br, bh]).astype(np.float32)
    bb = np.ascontiguousarray(np.broadcast_to(bcat, (128, U3)))
    uzr = np.concatenate([Uz, Ur], axis=1).astype(np.float16)
    uh = np.asarray(Uh).astype(np.float16)
    eye8 = np.eye(BS, dtype=np.float32)
    ones4 = np.zeros((128, BS), dtype=np.float32)
    for g in range(4):
        for b in range(BS):
            ones4[32 * g + b, b] = 1.0

    nc = bacc.Bacc("TRN2", target_bir_lowering=False, debug=False,
                   num_devices=N_CORES)
    build(nc, T)

    in_maps = []
    for c in range(N_CORES):
        xc = x[c * BS:(c + 1) * BS]               # [BS, T, D]
        xT = np.ascontiguousarray(xc.reshape(BS * T, D).T)  # [D, BS*T]
        in_maps.append({
            "xT": xT, "wcat": wcat, "bb": bb, "uzr": uzr, "uh": uh,
            "eye8r": eye8, "eye8f": eye8, "ones4": ones4,
        })
    return nc, in_maps


def assemble(results):
    outs = []
    for c in range(N_CORES):
        o = results[c]["out"]                     # [T, BS, U]
        outs.append(np.ascontiguousarray(o.transpose(1, 0, 2)))
    return np.concatenate(outs, axis=0)           # [B, T, U]


def kernel(inputs, Wz, Uz, bz, Wr, Ur, br, Wh, Uh, bh, _T=None):
    T = inputs.shape[1] if _T is None else _T
    nc, in_maps = prepare(inputs, Wz, Uz, bz, Wr, Ur, br, Wh, Uh, bh, T)
    res = run_bass_kernel_spmd(nc, in_maps, list(range(N_CORES)))
    return assemble(res.results)



# revision 6
# speedup vs baseline: 4.6360x; 4.6360x over previous
"""Trainium2 Bass kernel for CustomGRU (B=64, T=512, D=512, U=1024).

Sharding: data-parallel over batch across 8 NeuronCores (8 rows each),
weights replicated. Everything runs U-major ("transposed") so no PE
transposes are needed anywhere:

  h^T lives as a [128, 64] tile, col = k*8 + b, value = h[b, 128k + p].

Per step t the three gate pre-activations are computed with the weight
chunks STATIONARY ([128,128] fp16 tiles of Uz/Ur/Uh and Wz/Wr/Wh) and
h^T / x_t^T / (r*h)^T chunks MOVING ([128, 8] fp16 slices).  Each
matmul emits an [128-part, 8-free] PSUM tile slice, so the PE charge is
8 rows per matmul; a gate (K=1024, width 1024) is 64 such matmuls
accumulated over 8 K-chunks into 8 column groups of one PSUM tile.

The input projections x_t @ W (K=512, 4 chunks) are folded into the
same PSUM accumulations (start=True on the first chunk) and are issued
one step ahead, off the recurrence critical path.  x^T is
host-pre-transposed fp16 and DMA'd in 16-step blocks.

Per-step critical path: r-matmuls -> sigmoid(r) -> r*h (DVE) ->
hh-matmuls -> tanh -> z'*hh -> h = z*h_prev + z'*hh.  z, z' = 1-z
(sigmoid with scale=-1) and a = z*h_prev run off-path.  h is kept in
fp32 (master, [128,64] slices of a 16-step staging tile DMA'd to DRAM)
plus an fp16 copy for the next step's moving operands; the two adds
share inputs so the fp16 one unblocks the next step immediately.

Host assemble un-transposes [T,128,64] U-major output to [B,T,U].
"""
import sys

if "/opt/trn_rl_repo" not in sys.path:
    sys.path.insert(0, "/opt/trn_rl_repo")

import numpy as np
from contextlib import ExitStack

import concourse.bass as bass
import concourse.bacc as bacc
import concourse.tile as tile
from concourse import mybir
from concourse.bass_utils import run_bass_kernel_spmd

F32 = mybir.dt.float32
F16 = mybir.dt.float16

N_CORES = 8
B = 64
BS = B // N_CORES  # 8 batch rows per core
D = 512
U = 1024
KU = U // 128      # 8 U-contraction chunks
KD = D // 128      # 4 D-contraction chunks
MS = U // 128      # 8 output u-slices per gate
TB = 16            # time-block for x loads / output stores
SIG = mybir.ActivationFunctionType.Sigmoid
TANH = mybir.ActivationFunctionType.Tanh


def build(nc, T, has_bias=False):
    assert T % TB == 0
    NB = T // TB

    # ---- DRAM I/O (per-core) ----
    # xt[blk][p, i*32 + kc*8 + b] = x[b, blk*TB+i, kc*128+p]
    xt_d = nc.dram_tensor("xt", [NB, 128, TB * KD * BS], F16,
                          kind="ExternalInput")
    # w[p, ((g*KD + kc)*MS + m)*128 + j] = Wg[kc*128+p, m*128+j]
    w_d = nc.dram_tensor("w", [128, 3 * KD * MS * 128], F16,
                         kind="ExternalInput")
    # u[p, ((g*KU + kc)*MS + m)*128 + j] = Ug[kc*128+p, m*128+j]
    u_d = nc.dram_tensor("u", [128, 3 * KU * MS * 128], F16,
                         kind="ExternalInput")
    # bias chunks: row 0 of chunk (g, m) holds b_g[m*128 : (m+1)*128]
    bias_d = nc.dram_tensor("bias", [128, 3 * MS * 128], F16,
                            kind="ExternalInput")
    # U-major output: out[blk][p, i*64 + k*8 + b] = h_t[b, k*128+p]
    out_d = nc.dram_tensor("out", [NB, 128, TB * KU * BS], F32,
                           kind="ExternalOutput")

    def u_sl(g, kc, m):
        off = ((g * KU + kc) * MS + m) * 128
        return u_t[:, off:off + 128]

    def w_sl(g, kc, m):
        off = ((g * KD + kc) * MS + m) * 128
        return w_t[:, off:off + 128]

    with tile.TileContext(nc) as tc, ExitStack() as ctx:
        wpool = ctx.enter_context(tc.tile_pool(name="w", bufs=1))
        u_t = wpool.tile([128, 3 * KU * MS * 128], F16)
        nc.sync.dma_start(u_t[:], u_d[:])
        w_t = wpool.tile([128, 3 * KD * MS * 128], F16)
        nc.sync.dma_start(w_t[:], w_d[:])
        if has_bias:
            bias_t = wpool.tile([128, 3 * MS * 128], F16)
            nc.sync.dma_start(bias_t[:], bias_d[:])
            ones8 = wpool.tile([128, BS], F16)
            nc.any.memzero(ones8[:])
            nc.vector.memset(ones8[0:1, :], 1.0)

        hT0 = wpool.tile([128, KU * BS], F16)
        nc.any.memzero(hT0[:])
        h320 = wpool.tile([128, KU * BS], F32)
        nc.any.memzero(h320[:])

        xpool = ctx.enter_context(tc.tile_pool(name="xt", bufs=2))
        spool = ctx.enter_context(tc.tile_pool(name="stage", bufs=2))
        hpool = ctx.enter_context(tc.tile_pool(name="h", bufs=2))
        gpool = ctx.enter_context(tc.tile_pool(name="g", bufs=2))
        # one full PSUM bank (2KB/partition) per step: start=True on the
        # first matmul zeroes the whole bank (ZERO_REGION granularity), so
        # every other matmul uses start=False and each region's first write
        # lands on pending-zero bytes (overwrite), later ones accumulate.
        pp = ctx.enter_context(tc.tile_pool(name="ps", bufs=2, space="PSUM"))

        def x_mms(ps, xt_t, i):
            """Bias + x-projection partial sums for one step (off-path).

            ps cols 0:64 = r, 64:128 = z, 128:192 = hh."""
            for g, coff in ((1, 0), (0, 64), (2, 128)):
                for m in range(MS):
                    sl = ps[:, coff + m * BS: coff + (m + 1) * BS]
                    first = g == 1 and m == 0
                    if has_bias:
                        nc.tensor.matmul(sl, bias_t[:, (g * MS + m) * 128:
                                                    (g * MS + m + 1) * 128],
                                         ones8[:], start=first, stop=False,
                                         skip_group_check=True)
                    for kc in range(KD):
                        mv = xt_t[:, (i * KD + kc) * BS:(i * KD + kc + 1) * BS]
                        nc.tensor.matmul(sl, w_sl(g, kc, m), mv,
                                         start=(first and kc == 0
                                                and not has_bias),
                                         stop=False, skip_group_check=True)

        def h_mms(ps, coff, g, mv_t, stop):
            for m in range(MS):
                sl = ps[:, coff + m * BS: coff + (m + 1) * BS]
                for kc in range(KU):
                    nc.tensor.matmul(sl, u_sl(g, kc, m),
                                     mv_t[:, kc * BS:(kc + 1) * BS],
                                     start=False, stop=(stop and kc == KU - 1),
                                     skip_group_check=True)

        # block 0 x tile + step-0 partials
        xt_t = xpool.tile([128, TB * KD * BS], F16, tag="xt")
        nc.sync.dma_start(xt_t[:], xt_d[0].squeeze())
        ps = pp.tile([128, 512], F32, tag="ps")
        x_mms(ps, xt_t, 0)

        hT_prev, h32_prev = hT0, h320
        stage = None
        for t in range(T):
            blk, i = divmod(t, TB)
            if i == 0:
                if blk + 1 < NB:
                    xt_next = xpool.tile([128, TB * KD * BS], F16, tag="xt")
                    nc.sync.dma_start(xt_next[:], xt_d[blk + 1].squeeze())
                stage = spool.tile([128, TB * KU * BS], F32, tag="st")

            # ---- critical path: r -> sigmoid -> r*h -> hh -> tanh ----
            h_mms(ps, 0, 1, hT_prev, stop=True)             # r
            h_mms(ps, 64, 0, hT_prev, stop=True)            # z
            r16 = gpool.tile([128, KU * BS], F16, tag="r")
            nc.scalar.activation(r16[:], ps[:, 0:64], SIG)
            rh16 = gpool.tile([128, KU * BS], F16, tag="rh")
            nc.vector.tensor_mul(rh16[:], r16[:], hT_prev[:])

            z16 = gpool.tile([128, KU * BS], F16, tag="z")
            nc.scalar.activation(z16[:], ps[:, 64:128], SIG)
            zp16 = gpool.tile([128, KU * BS], F16, tag="zp")
            nc.scalar.activation(zp16[:], ps[:, 64:128], SIG, scale=-1.0)
            a32 = gpool.tile([128, KU * BS], F32, tag="a")
            nc.vector.tensor_mul(a32[:], z16[:], h32_prev[:])

            # next step's x/bias partials (PE fills idle time before hh)
            if t + 1 < T:
                ps_n = pp.tile([128, 512], F32, tag="ps")
                i2 = (t + 1) % TB
                x_mms(ps_n, xt_t if i2 else xt_next, i2)

            h_mms(ps, 128, 2, rh16, stop=True)              # hh
            hh16 = gpool.tile([128, KU * BS], F16, tag="hh")
            nc.scalar.activation(hh16[:], ps[:, 128:192], TANH)

            # h = z*h_prev + (1-z)*hh ; fp16 copy first to unblock t+1
            b16 = gpool.tile([128, KU * BS], F16, tag="b")
            nc.vector.tensor_mul(b16[:], zp16[:], hh16[:])
            hT_new = hpool.tile([128, KU * BS], F16, tag="hT")
            nc.vector.tensor_add(hT_new[:], a32[:], b16[:])
            st_sl = stage[:, i * 64:(i + 1) * 64]
            nc.vector.tensor_add(st_sl, a32[:], b16[:])

            if i == TB - 1:
                nc.sync.dma_start(out_d[blk].squeeze(), stage[:])
                xt_t = xt_next if blk + 1 < NB else None
            if t + 1 < T:
                ps = ps_n
            hT_prev, h32_prev = hT_new, st_sl

    nc.compile()
    return nc


def prepare(inputs, Wz, Uz, bz, Wr, Ur, br, Wh, Uh, bh, T):
    """Build the Bass program and the per-core input maps."""
    x = np.asarray(inputs, dtype=np.float32)[:, :T, :]
    NB = T // TB

    def w_pack(w):   # [D, U] -> [128, KD*MS*128]
        return np.ascontiguousarray(
            np.asarray(w, np.float32).reshape(KD, 128, MS, 128)
            .transpose(1, 0, 2, 3).reshape(128, KD * MS * 128)
        ).astype(np.float16)

    def u_pack(u):   # [U, U] -> [128, KU*MS*128]
        return np.ascontiguousarray(
            np.asarray(u, np.float32).reshape(KU, 128, MS, 128)
            .transpose(1, 0, 2, 3).reshape(128, KU * MS * 128)
        ).astype(np.float16)

    w_host = np.concatenate([w_pack(Wz), w_pack(Wr), w_pack(Wh)], axis=1)
    u_host = np.concatenate([u_pack(Uz), u_pack(Ur), u_pack(Uh)], axis=1)

    has_bias = any(float(np.abs(np.asarray(b)).max()) != 0.0
                   for b in (bz, br, bh))
    bias_host = np.zeros((128, 3 * MS * 128), np.float16)
    for g, b in enumerate((bz, br, bh)):
        bias_host[0, g * MS * 128:(g + 1) * MS * 128] = (
            np.asarray(b, np.float32).astype(np.float16))

    nc = bacc.Bacc("TRN2", target_bir_lowering=False, debug=False,
                   num_devices=N_CORES)
    build(nc, T, has_bias=has_bias)

    in_maps = []
    for c in range(N_CORES):
        xc = x[c * BS:(c + 1) * BS]               # [BS, T, D]
        # (b, blk, i, kc, p) -> (blk, p, i, kc, b)
        xt = np.ascontiguousarray(
            xc.reshape(BS, NB, TB, KD, 128).transpose(1, 4, 2, 3, 0)
            .reshape(NB, 128, TB * KD * BS)
        ).astype(np.float16)
        in_maps.append({
            "xt": xt, "w": w_host, "u": u_host, "bias": bias_host,
        })
    return nc, in_maps


def assemble(results):
    outs = []
    for c in range(N_CORES):
        o = results[c]["out"]                     # [NB, 128, TB*KU*BS]
        NB = o.shape[0]
        # (blk, p, i, k, b) -> (b, blk, i, k, p)
        outs.append(
            o.reshape(NB, 128, TB, KU, BS).transpose(4, 0, 2, 3, 1)
            .reshape(BS, NB * TB, U)
        )
    return np.ascontiguousarray(np.concatenate(outs, axis=0))  # [B, T, U]


def kernel(inputs, Wz, Uz, bz, Wr, Ur, br, Wh, Uh, bh, _T=None):
    T = inputs.shape[1] if _T is None else _T
    nc, in_maps = prepare(inputs, Wz, Uz, bz, Wr, Ur, br, Wh, Uh, bh, T)
    res = run_bass_kernel_spmd(nc, in_maps, list(range(N_CORES)))
    return assemble(res.results)


# revision 21
# speedup vs baseline: 44.4796x; 9.5943x over previous
"""Trainium2 Bass kernel for CustomGRU (B=64, T=512, D=512, U=1024).

Sharding: data-parallel over batch across 8 NeuronCores (8 rows each),
weights replicated. Everything runs U-major ("transposed") so no PE
transposes are needed anywhere:

  h^T lives as a [128, 64] tile, col = k*8 + b, value = h[b, 128k + p].

Per step t the three gate pre-activations are computed with the weight
chunks STATIONARY ([128,128] fp16 tiles of Uz/Ur/Uh and Wz/Wr/Wh) and
h^T / x_t^T / (r*h)^T chunks MOVING ([128, 8] fp16 slices).  The cost
of a matmul is its output free size (8 rows) - the contraction
(partition) dim and the stationary load are free - so a gate (K=1024,
width 1024) is 64 cheap matmuls accumulated over 8 K-chunks into 8
column groups of a PSUM bank.  The x_t @ W projections (4 K-chunks)
are folded into the same accumulations one step ahead, off the
recurrence critical path.

PSUM: r, z and hh each get a WHOLE bank (2KB/partition) per step,
double-buffered (6 of 8 banks).  Dependency tracking is tile-granular,
so separate banks keep sigmoid(r) waiting only on the r matmuls and
keep the hh matmuls from serializing behind sigmoid(z) reads.
start=True on a bank's first matmul zeroes the whole bank (pending-
zero at ZERO_REGION granularity); all other matmuls use start=False
(each region's first write lands on pending-zero bytes, overwriting).

Per-step critical path: r-mms -> sigmoid(r) -> r*h (DVE) -> hh-mms ->
tanh -> (1-z)*hh -> h = z*h_prev + (1-z)*hh.  z, z' = 1-z (sigmoid
with scale=-1) and a = z*h_prev run off-path; the z matmuls and next
step's x matmuls fill the PE while sigmoid(r)/r*h are in flight.  h is
kept fp32 (staging tile, DMA'd per 16-step block) plus an fp16 copy
whose add runs first to unblock the next step's matmuls.

Host assemble un-transposes [T,128,64] U-major output to [B,T,U].
"""
import sys

if "/opt/trn_rl_repo" not in sys.path:
    sys.path.insert(0, "/opt/trn_rl_repo")

import numpy as np
from contextlib import ExitStack

import concourse.bass as bass
import concourse.bacc as bacc
import concourse.tile as tile
from concourse import mybir
from concourse.bass_utils import run_bass_kernel_spmd

F32 = mybir.dt.float32
F16 = mybir.dt.float16

N_CORES = 8
B = 64
BS = B // N_CORES  # 8 batch rows per core
D = 512
U = 1024
KU = U // 128      # 8 U-contraction chunks
KD = D // 128      # 4 D-contraction chunks
MS = U // 128      # 8 output u-slices per gate
TB = 16            # time-block for x loads / output stores
SIG = mybir.ActivationFunctionType.Sigmoid
TANH = mybir.ActivationFunctionType.Tanh


def build(nc, T, has_bias=False):
    assert T % TB == 0
    NB = T // TB

    # ---- DRAM I/O (per-core) ----
    # xt[blk][p, (i*KD + kc)*8 + b] = x[b, blk*TB+i, kc*128+p]
    xt_d = nc.dram_tensor("xt", [NB, 128, TB * KD * BS], F16,
                          kind="ExternalInput")
    # w[p, ((g*KD + kc)*MS + m)*128 + j] = Wg[kc*128+p, m*128+j]
    w_d = nc.dram_tensor("w", [128, 3 * KD * MS * 128], F16,
                         kind="ExternalInput")
    u_d = nc.dram_tensor("u", [128, 3 * KU * MS * 128], F16,
                         kind="ExternalInput")
    bias_d = nc.dram_tensor("bias", [128, 3 * MS * 128], F16,
                            kind="ExternalInput")
    # U-major output: out[blk][p, i*64 + k*8 + b] = h_t[b, k*128+p]
    out_d = nc.dram_tensor("out", [NB, 128, TB * KU * BS], F32,
                           kind="ExternalOutput")

    def u_sl(g, kc, m):
        off = ((g * KU + kc) * MS + m) * 128
        return u_t[:, off:off + 128]

    def w_sl(g, kc, m):
        off = ((g * KD + kc) * MS + m) * 128
        return w_t[:, off:off + 128]

    with tile.TileContext(nc) as tc, ExitStack() as ctx:
        wpool = ctx.enter_context(tc.tile_pool(name="w", bufs=1))
        u_t = wpool.tile([128, 3 * KU * MS * 128], F16)
        nc.sync.dma_start(u_t[:], u_d[:])
        w_t = wpool.tile([128, 3 * KD * MS * 128], F16)
        nc.sync.dma_start(w_t[:], w_d[:])
        if has_bias:
            bias_t = wpool.tile([128, 3 * MS * 128], F16)
            nc.sync.dma_start(bias_t[:], bias_d[:])
            ones8 = wpool.tile([128, BS], F16)
            nc.any.memzero(ones8[:])
            nc.vector.memset(ones8[0:1, :], 1.0)

        hT0 = wpool.tile([128, KU * BS], F16)
        nc.any.memzero(hT0[:])
        h320 = wpool.tile([128, KU * BS], F32)
        nc.any.memzero(h320[:])

        xpool = ctx.enter_context(tc.tile_pool(name="xt", bufs=2))
        spool = ctx.enter_context(tc.tile_pool(name="stage", bufs=2))
        hpool = ctx.enter_context(tc.tile_pool(name="h", bufs=2))
        gpool = ctx.enter_context(tc.tile_pool(name="g", bufs=2))
        prp = ctx.enter_context(tc.tile_pool(name="pr", bufs=2, space="PSUM"))
        pzp = ctx.enter_context(tc.tile_pool(name="pz", bufs=2, space="PSUM"))
        php = ctx.enter_context(tc.tile_pool(name="ph", bufs=2, space="PSUM"))

        def x_mms(psr, psz, psh, xt_t, i):
            """Bias + x-projection partials for one step (off-path)."""
            for g, ps in ((1, psr), (0, psz), (2, psh)):
                for m in range(MS):
                    sl = ps[:, m * BS:(m + 1) * BS]
                    first = m == 0
                    if has_bias:
                        nc.tensor.matmul(sl, bias_t[:, (g * MS + m) * 128:
                                                    (g * MS + m + 1) * 128],
                                         ones8[:], start=first, stop=False,
                                         skip_group_check=True)
                    for kc in range(KD):
                        o = (i * KD + kc) * BS
                        nc.tensor.matmul(sl, w_sl(g, kc, m), xt_t[:, o:o + BS],
                                         start=(first and kc == 0
                                                and not has_bias),
                                         stop=False, skip_group_check=True)

        def h_mms(ps, g, mv_t):
            for m in range(MS):
                sl = ps[:, m * BS:(m + 1) * BS]
                for kc in range(KU):
                    nc.tensor.matmul(sl, u_sl(g, kc, m),
                                     mv_t[:, kc * BS:(kc + 1) * BS],
                                     start=False, stop=(kc == KU - 1),
                                     skip_group_check=True)

        # block 0 x tile + step-0 partials
        xt_t = xpool.tile([128, TB * KD * BS], F16, tag="xt")
        nc.sync.dma_start(xt_t[:], xt_d[0].squeeze())
        psr = prp.tile([128, 512], F32, tag="pr")
        psz = pzp.tile([128, 512], F32, tag="pz")
        psh = php.tile([128, 512], F32, tag="ph")
        x_mms(psr, psz, psh, xt_t, 0)

        hT_prev, h32_prev = hT0, h320
        stage = None
        for t in range(T):
            blk, i = divmod(t, TB)
            if i == 0:
                if blk + 1 < NB:
                    xt_next = xpool.tile([128, TB * KD * BS], F16, tag="xt")
                    nc.sync.dma_start(xt_next[:], xt_d[blk + 1].squeeze())
                stage = spool.tile([128, TB * KU * BS], F32, tag="st")

            # ---- critical path: r -> sigmoid -> r*h -> hh -> tanh ----
            h_mms(psr, 1, hT_prev)                          # r
            r16 = gpool.tile([128, KU * BS], F16, tag="r")
            nc.scalar.activation(r16[:], psr[:, 0:64], SIG)
            rh16 = gpool.tile([128, KU * BS], F16, tag="rh")
            nc.vector.tensor_mul(rh16[:], r16[:], hT_prev[:])

            # off-path work that fills PE/ACT while sigmoid(r), r*h fly
            h_mms(psz, 0, hT_prev)                          # z
            z16 = gpool.tile([128, KU * BS], F16, tag="z")
            nc.scalar.activation(z16[:], psz[:, 0:64], SIG)
            zp16 = gpool.tile([128, KU * BS], F16, tag="zp")
            nc.scalar.activation(zp16[:], psz[:, 0:64], SIG, scale=-1.0)
            # z*h_prev in both precisions: fp16 for the recurrence path
            # (all-fp16 combine -> DVE 2x), fp32 for the staged output
            a16 = gpool.tile([128, KU * BS], F16, tag="a16")
            nc.vector.tensor_mul(a16[:], z16[:], hT_prev[:])

            h_mms(psh, 2, rh16)                             # hh
            hh16 = gpool.tile([128, KU * BS], F16, tag="hh")
            nc.scalar.activation(hh16[:], psh[:, 0:64], TANH)

            if t + 1 < T:
                psr_n = prp.tile([128, 512], F32, tag="pr")
                psz_n = pzp.tile([128, 512], F32, tag="pz")
                psh_n = php.tile([128, 512], F32, tag="ph")
                i2 = (t + 1) % TB
                x_mms(psr_n, psz_n, psh_n, xt_t if i2 else xt_next, i2)

            # h = z*h_prev + (1-z)*hh ; fp16 add first to unblock t+1
            b16 = gpool.tile([128, KU * BS], F16, tag="b")
            nc.vector.tensor_mul(b16[:], zp16[:], hh16[:])
            hT_new = hpool.tile([128, KU * BS], F16, tag="hT")
            nc.vector.tensor_add(hT_new[:], a16[:], b16[:])
            a32 = gpool.tile([128, KU * BS], F32, tag="a")
            nc.vector.tensor_mul(a32[:], z16[:], h32_prev[:])
            st_sl = stage[:, i * 64:(i + 1) * 64]
            nc.vector.tensor_add(st_sl, a32[:], b16[:])

            if i == TB - 1:
                nc.sync.dma_start(out_d[blk].squeeze(), stage[:])
                xt_t = xt_next if blk + 1 < NB else None
            if t + 1 < T:
                psr, psz, psh = psr_n, psz_n, psh_n
            hT_prev, h32_prev = hT_new, st_sl

    nc.compile()
    return nc


def prepare(inputs, Wz, Uz, bz, Wr, Ur, br, Wh, Uh, bh, T):
    """Build the Bass program and the per-core input maps."""
    x = np.asarray(inputs, dtype=np.float32)[:, :T, :]
    NB = T // TB

    def w_pack(w):   # [D, U] -> [128, KD*MS*128]
        return np.ascontiguousarray(
            np.asarray(w, np.float32).reshape(KD, 128, MS, 128)
            .transpose(1, 0, 2, 3).reshape(128, KD * MS * 128)
        ).astype(np.float16)

    def u_pack(u):   # [U, U] -> [128, KU*MS*128]
        return np.ascontiguousarray(
            np.asarray(u, np.float32).reshape(KU, 128, MS, 128)
            .transpose(1, 0, 2, 3).reshape(128, KU * MS * 128)
        ).astype(np.float16)

    w_host = np.concatenate([w_pack(Wz), w_pack(Wr), w_pack(Wh)], axis=1)
    u_host = np.concatenate([u_pack(Uz), u_pack(Ur), u_pack(Uh)], axis=1)

    has_bias = any(float(np.abs(np.asarray(b)).max()) != 0.0
                   for b in (bz, br, bh))
    bias_host = np.zeros((128, 3 * MS * 128), np.float16)
    for g, b in enumerate((bz, br, bh)):
        bias_host[0, g * MS * 128:(g + 1) * MS * 128] = (
            np.asarray(b, np.float32).astype(np.float16))

    nc = bacc.Bacc("TRN2", target_bir_lowering=False, debug=False,
                   num_devices=N_CORES)
    build(nc, T, has_bias=has_bias)

    in_maps = []
    for c in range(N_CORES):
        xc = x[c * BS:(c + 1) * BS]               # [BS, T, D]
        # (b, blk, i, kc, p) -> (blk, p, i, kc, b)
        xt = np.ascontiguousarray(
            xc.reshape(BS, NB, TB, KD, 128).transpose(1, 4, 2, 3, 0)
            .reshape(NB, 128, TB * KD * BS)
        ).astype(np.float16)
        in_maps.append({
            "xt": xt, "w": w_host, "u": u_host, "bias": bias_host,
        })
    return nc, in_maps


def assemble(results):
    outs = []
    for c in range(N_CORES):
        o = results[c]["out"]                     # [NB, 128, TB*KU*BS]
        NB = o.shape[0]
        # (blk, p, i, k, b) -> (b, blk, i, k, p)
        outs.append(
            o.reshape(NB, 128, TB, KU, BS).transpose(4, 0, 2, 3, 1)
            .reshape(BS, NB * TB, U)
        )
    return np.ascontiguousarray(np.concatenate(outs, axis=0))  # [B, T, U]


def kernel(inputs, Wz, Uz, bz, Wr, Ur, br, Wh, Uh, bh, _T=None):
    T = inputs.shape[1] if _T is None else _T
    nc, in_maps = prepare(inputs, Wz, Uz, bz, Wr, Ur, br, Wh, Uh, bh, T)
    res = run_bass_kernel_spmd(nc, in_maps, list(range(N_CORES)))
    return assemble(res.results)
